# revision 1
# baseline (speedup 1.0000x reference)
"""Distributed multi-head causal attention for TRN2, 8 NeuronCores.

Strategy (tensor-parallel over heads + AllToAll re-shard for the output
projection):
  - Each core owns 2 of the 16 heads. It computes Q,K,V projections for its
    heads over the full sequence (both batches), applies RoPE, and computes
    causal softmax(QK^T/sqrt(hd)) @ V for its heads.
  - Everything on-chip is laid out TRANSPOSED: qT/kT are [hd, B*S], scores are
    [k, q], attention output is [hd, q].  This avoids all transposes:
      scoresT = kT_block.T @ qT        (lhsT=kT block, rhs=qT)
      aoT     = v_block.T  @ pT        (lhsT=v natural [k,hd], rhs=pT [k,q])
    Softmax denominator (sum over k = partition axis) comes from a ones-matmul
    (lhsT=ones [128,128]) that also broadcasts the sum across partitions.
    exp() is computed WITHOUT max subtraction (max |score| ~ 6, safe in f32).
  - Per-batch AllToAll swaps head-sharding for sequence-sharding (overlapped
    with the other batch's attention / wo compute): each core ends with all
    16 heads for its 256-position slice of each batch, then computes its
    slice of the wo projection: outT = woT_chunk.T @ attnT (output
    transposed; host transposes back).
  - Compute dtype: bf16 matmul operands, f32 PSUM accumulation, f32 softmax.

Host-side prep casts inputs to bf16 and pre-transposes x/wo; host-side
assembly transposes/concats per-core outputs.  No host arithmetic.
"""
import math

import ml_dtypes
import numpy as np

import concourse.bass as bass
import concourse.mybir as mybir
from concourse import bacc
from concourse.tile import TileContext

F32 = mybir.dt.float32
BF16 = mybir.dt.bfloat16

N_CORES = 8
CORE_IDS = list(range(N_CORES))
B = 2
S = 2048
D = 2048
H = 16
HD = 128  # head dim
HPC = H // N_CORES  # heads per core = 2
BS = B * S  # 4096
NB = S // 512  # 4 q-free-blocks per batch
NK = S // 128  # 16 k-blocks per batch
SCHUNK = S // N_CORES  # 256 positions per core per batch
INV_SQRT_HD = 1.0 / math.sqrt(HD)

# stream_shuffle mask: swap adjacent partitions within each 32-group
PAIR_SWAP = [i ^ 1 for i in range(32)]


def build():
    nc = bacc.Bacc(None, num_devices=N_CORES)

    xt = nc.declare_dram_parameter("xt", [2 * B, 128, 16, 1024], BF16, isOutput=False)
    wqt = nc.declare_dram_parameter("wqt", [128, 16, HPC * HD], BF16, isOutput=False)
    wkt = nc.declare_dram_parameter("wkt", [128, 16, HPC * HD], BF16, isOutput=False)
    wvt = nc.declare_dram_parameter("wvt", [128, 16, HPC * HD], BF16, isOutput=False)
    wot = nc.declare_dram_parameter("wot", [128, 16, D], BF16, isOutput=False)
    cgrid = nc.declare_dram_parameter("cgrid", [HD, S], F32, isOutput=False)
    sgrid = nc.declare_dram_parameter("sgrid", [HD, S], F32, isOutput=False)
    masks = nc.declare_dram_parameter("masks", [HD, 2, 1024], BF16, isOutput=False)
    out_ext = nc.declare_dram_parameter("out", [D, B * SCHUNK], F32, isOutput=True)

    bnc_in = [
        nc.dram_tensor(f"bounce_in{b}", [N_CORES, HPC * HD, SCHUNK], BF16)
        for b in range(B)
    ]
    bnc_out = [
        nc.dram_tensor(f"bounce_out{b}", [N_CORES, HPC * HD, SCHUNK], BF16)
        for b in range(B)
    ]

    bar_in = nc.dram_tensor("bar_in", [1], F32)
    bar_out = nc.dram_tensor("bar_out", [N_CORES], F32, addr_space="Shared")

    with TileContext(nc) as tc:
        with (
            tc.tile_pool(name="persist", bufs=1) as persist,
            tc.tile_pool(name="tmp", bufs=4) as tmp,
        ):
            # ---------------- persistent SBUF tensors ----------------
            mask_sb = persist.tile([128, 2, 1024], BF16, tag="mask")
            ones_sb = persist.tile([128, 128], BF16, tag="ones")
            nc.vector.memset(ones_sb, 1.0)

            # qT/kT per head: [hd=128, BS] bf16 (post-RoPE).
            # v per head: [128, BS] bf16, chunk ik at cols [128*ik,128*(ik+1))
            # holding v rows (k) on partitions, hd on free.
            q_sb = [persist.tile([128, BS], BF16, tag=f"q{h}", name=f"q_sb{h}") for h in range(HPC)]
            k_sb = [persist.tile([128, BS], BF16, tag=f"k{h}", name=f"k_sb{h}") for h in range(HPC)]
            v_sb = [persist.tile([128, BS], BF16, tag=f"v{h}", name=f"v_sb{h}") for h in range(HPC)]

            # ---------------- phase 1: QKV projections + RoPE ----------------
            with (
                tc.tile_pool(name="p1w", bufs=1) as p1w,
                tc.tile_pool(name="xt_pool", bufs=2) as xt_pool,
                tc.tile_pool(name="p1psum", bufs=1, space="PSUM") as p1psum,
                tc.tile_pool(name="p1psumv", bufs=2, space="PSUM") as p1psumv,
            ):
                wq_sb = p1w.tile([128, 16, HPC * HD], BF16, tag="wq")
                wk_sb = p1w.tile([128, 16, HPC * HD], BF16, tag="wk")
                wv_sb = p1w.tile([128, 16, HPC * HD], BF16, tag="wv")
                nc.gpsimd.dma_start(out=wq_sb, in_=wqt[:, :, :])
                nc.gpsimd.dma_start(out=wk_sb, in_=wkt[:, :, :])
                nc.gpsimd.dma_start(out=wv_sb, in_=wvt[:, :, :])
                nc.gpsimd.dma_start(out=mask_sb, in_=masks[:, :, :])
                # dummy AllGather: absorbs cross-core NEFF-launch skew early,
                # so the later AllToAlls see aligned peers
                nc.gpsimd.collective_compute(
                    "AllGather",
                    mybir.AluOpType.bypass,
                    replica_groups=[CORE_IDS],
                    ins=[bar_in[:]],
                    outs=[bar_out[:]],
                )
                cg_sb = p1w.tile([128, S], F32, tag="cg")
                sg_sb = p1w.tile([128, S], F32, tag="sg")

                for half in range(2 * B):  # half-batches of 1024 positions
                    b, hf = divmod(half, 2)
                    coff = b * S + hf * 1024  # column offset in [D, BS]
                    poff = hf * 1024  # position offset within batch (grids)
                    xt_sb = xt_pool.tile([128, 16, 1024], BF16, tag="xt")
                    for q4 in range(4):
                        eng = nc.sync if q4 % 2 == 0 else nc.scalar
                        eng.dma_start(
                            out=xt_sb[:, q4 * 4:(q4 + 1) * 4, :],
                            in_=xt[half, :, q4 * 4:(q4 + 1) * 4, :],
                        )
                    if half == 0:
                        # grids needed only at RoPE time; issue after half-0
                        nc.scalar.dma_start(out=cg_sb, in_=cgrid[:, :])
                        nc.scalar.dma_start(out=sg_sb, in_=sgrid[:, :])

                    # Q, K for both heads: psum [hd, 512] accumulated over d_in
                    for j2 in range(2):
                        ps = {}
                        for kind, w in (("q", wq_sb), ("k", wk_sb)):
                            for h in range(HPC):
                                p = p1psum.tile([128, 512], F32, tag=f"qk{kind}{h}")
                                ps[(kind, h)] = p
                                for i in range(16):
                                    nc.tensor.matmul(
                                        p,
                                        w[:, i, h * HD:(h + 1) * HD],
                                        xt_sb[:, i, j2 * 512:(j2 + 1) * 512],
                                        start=(i == 0),
                                        stop=(i == 15),
                                    )
                        # RoPE: out = t*cos + pairswap(t)*sin_signed  (DVE only)
                        gcol = slice(poff + j2 * 512, poff + (j2 + 1) * 512)
                        ocol = slice(coff + j2 * 512, coff + (j2 + 1) * 512)
                        for kind, dst in (("q", q_sb), ("k", k_sb)):
                            for h in range(HPC):
                                p = ps[(kind, h)]
                                m1 = tmp.tile([128, 512], F32, tag="rope_m1")
                                nc.vector.tensor_mul(m1, p, cg_sb[:, gcol])
                                sh = tmp.tile([128, 512], F32, tag="rope_sh")
                                nc.vector.stream_shuffle(sh, p, PAIR_SWAP)
                                nc.vector.tensor_mul(sh, sh, sg_sb[:, gcol])
                                nc.vector.tensor_add(dst[h][:, ocol], m1, sh)

                    # V for both heads: psum [s=128, 2*HD] accumulated over d_in
                    for s2 in range(8):
                        pv = p1psumv.tile([128, HPC * HD], F32, tag="v")
                        for i in range(16):
                            nc.tensor.matmul(
                                pv,
                                xt_sb[:, i, s2 * 128:(s2 + 1) * 128],
                                wv_sb[:, i, :],
                                start=(i == 0),
                                stop=(i == 15),
                            )
                        sc = hf * 8 + s2
                        ccol = slice((b * NK + sc) * 128, (b * NK + sc + 1) * 128)
                        for h in range(HPC):
                            nc.scalar.copy(
                                out=v_sb[h][:, ccol], in_=pv[:, h * HD:(h + 1) * HD]
                            )

            # ---------------- phases 2+3 pools ----------------
            with (
                tc.tile_pool(name="p23", bufs=1) as p23,
                tc.tile_pool(name="ptile", bufs=6) as ptile,
                tc.tile_pool(name="p2psum", bufs=2, space="PSUM") as p2psum,
            ):
                wo_sb = p23.tile([128, 16, D], BF16, tag="wo")
                nc.scalar.dma_start(out=wo_sb, in_=wot[:, :, :])

                # ---------------- phase 2: attention (batch-major) ----------------
                for b in range(B):
                    for h in range(HPC):
                        for jq in range(NB):
                            po = p2psum.tile([128, 512], F32, tag="pv", bufs=1)
                            pden = p2psum.tile([128, 512], F32, tag="den", bufs=1)
                            nkb = 4 * jq + 4  # causal: k-blocks 0..4jq+3
                            qcol = slice(b * S + jq * 512, b * S + (jq + 1) * 512)
                            # q-columns 256-511 only (high diagonal pair)
                            qcol_hi = slice(b * S + jq * 512 + 256, b * S + (jq + 1) * 512)
                            d_prev = None
                            for e in range(nkb // 2):  # k-block pairs
                                hi = e == 2 * jq + 1  # high diagonal pair:
                                # blocks 4jq+2/4jq+3 only reach q >= 256
                                w = 256 if hi else 512
                                psc = p2psum.tile([128, 2 * w], F32, tag="sc", name="psc")
                                for u in range(2):
                                    ik = 2 * e + u
                                    nc.tensor.matmul(
                                        psc[:, u * w:(u + 1) * w],
                                        k_sb[h][:, b * S + ik * 128: b * S + (ik + 1) * 128],
                                        q_sb[h][:, qcol_hi if hi else qcol],
                                        start=True,
                                        stop=True,
                                    )
                                p_sb = ptile.tile([128, 2 * w], BF16, tag="p", name="p_sb")
                                nc.scalar.activation(
                                    out=p_sb,
                                    in_=psc,
                                    func=mybir.ActivationFunctionType.Exp,
                                    scale=INV_SQRT_HD,
                                )
                                if e >= 2 * jq:  # diagonal pair: causal 0/1 mask
                                    nc.vector.tensor_mul(
                                        p_sb, p_sb, mask_sb[:, e - 2 * jq, 0:2 * w]
                                    )
                                for u in range(2):
                                    ik = 2 * e + u
                                    vcol = slice((b * NK + ik) * 128, (b * NK + ik + 1) * 128)
                                    nc.tensor.matmul(
                                        po[:, 256:512] if hi else po,
                                        v_sb[h][:, vcol], p_sb[:, u * w:(u + 1) * w],
                                        start=(ik == 0), stop=(ik == nkb - 1),
                                        skip_group_check=True,
                                    )
                                d_sb = tmp.tile([128, 512], BF16, tag="dpair")
                                nc.vector.tensor_add(
                                    d_sb[:, 0:w], p_sb[:, 0:w], p_sb[:, w:2 * w]
                                )
                                if e < 2 * jq and e % 2 == 0:
                                    d_prev = d_sb  # defer: pair up with next
                                    continue
                                if e < 2 * jq:  # odd off-diagonal: fold 2 pairs
                                    dd = tmp.tile([128, 512], BF16, tag="dquad")
                                    nc.vector.tensor_add(dd, d_prev, d_sb)
                                    d_sb = dd
                                nc.tensor.matmul(
                                    pden[:, 256:512] if hi else pden,
                                    ones_sb, d_sb[:, 0:w],
                                    start=(e == (1 if jq > 0 else 0) and not hi),
                                    stop=hi,
                                    skip_group_check=True,
                                )
                            recip = tmp.tile([128, 512], F32, tag="recip")
                            nc.vector.reciprocal_approx_fast(out=recip, in_=pden)
                            ao = tmp.tile([128, 512], BF16, tag="ao")
                            nc.vector.tensor_mul(ao, po, recip)
                            st_eng = nc.gpsimd if b == 0 else nc.sync
                            for u in range(2):
                                st_eng.dma_start(
                                    out=bnc_in[b][2 * jq + u, h * HD:(h + 1) * HD, :],
                                    in_=ao[:, u * 256:(u + 1) * 256],
                                )
                # per-batch A2As; each fires as soon as its batch's stores land
                for b in range(B):
                    nc.gpsimd.collective_compute(
                        "AllToAll",
                        mybir.AluOpType.bypass,
                        replica_groups=[CORE_IDS],
                        ins=[bnc_in[b][:, :, :]],
                        outs=[bnc_out[b][:, :, :]],
                    )

                # ---------------- phase 3: output projection ----------------
                for b in range(B):
                    g_sb = p23.tile([128, 16, SCHUNK], BF16, tag=f"g{b}", name=f"g_sb{b}")
                    nc.scalar.dma_start(
                        out=g_sb,
                        in_=bnc_out[b].rearrange("j (u p) n -> p (j u) n", p=128),
                    )
                    for m in range(16):
                        pw = p2psum.tile([128, SCHUNK], F32, tag="wo", bufs=2)
                        for i in range(16):
                            nc.tensor.matmul(
                                pw,
                                wo_sb[:, i, m * 128:(m + 1) * 128],
                                g_sb[:, i, :],
                                start=(i == 0),
                                stop=(i == 15),
                            )
                        o_sb = tmp.tile([128, SCHUNK], F32, tag="o")
                        nc.vector.tensor_copy(out=o_sb, in_=pw)
                        nc.sync.dma_start(
                            out=out_ext[m * 128:(m + 1) * 128, b * SCHUNK:(b + 1) * SCHUNK],
                            in_=o_sb,
                        )

    nc.compile()
    return nc


def prep_inputs(x, freqs_cos, freqs_sin, wq, wk, wv, wo):
    """Host-side shard prep. Returns in_maps (list of 8 dicts)."""
    bf = ml_dtypes.bfloat16
    x = np.asarray(x, dtype=np.float32)
    xtf = x.reshape(BS, D).T.astype(bf)  # [D, BS]
    # partition-major pre-chunk: [half, p, chunk, n] -> fully sequential DMAs
    xt = np.ascontiguousarray(xtf.reshape(16, 128, 2 * B, 1024).transpose(2, 1, 0, 3))
    wot = np.ascontiguousarray(np.asarray(wo, np.float32).T.astype(bf).reshape(16, 128, D).transpose(1, 0, 2))
    cos = np.asarray(freqs_cos, np.float32)
    sin = np.asarray(freqs_sin, np.float32)
    cg = np.empty((HD, S), np.float32)
    sg = np.empty((HD, S), np.float32)
    cg[0::2] = cos.T
    cg[1::2] = cos.T
    sg[0::2] = -sin.T
    sg[1::2] = sin.T
    mk4 = np.zeros((4, HD, 512), np.float32)
    for t in range(4):
        kp = np.arange(HD)[:, None]
        qf = np.arange(512)[None, :]
        mk4[t] = (128 * t + kp <= qf).astype(np.float32)
    # mk[0]: low diagonal pair (blocks t0,t1) over full 512 q-cols;
    # mk[1][:, :512]: high pair (t2,t3) restricted to q-cols 256-511
    mk = np.zeros((2, HD, 1024), np.float32)
    mk[0][:, 0:512] = mk4[0]
    mk[0][:, 512:1024] = mk4[1]
    mk[1][:, 0:256] = mk4[2][:, 256:512]
    mk[1][:, 256:512] = mk4[3][:, 256:512]
    mk = np.ascontiguousarray(mk.astype(bf).transpose(1, 0, 2))

    in_maps = []
    for c in range(N_CORES):
        rows = slice(c * HPC * HD, (c + 1) * HPC * HD)
        in_maps.append({
            "xt": xt,
            "wqt": np.ascontiguousarray(np.asarray(wq, np.float32)[rows, :].T.astype(bf).reshape(16, 128, HPC * HD).transpose(1, 0, 2)),
            "wkt": np.ascontiguousarray(np.asarray(wk, np.float32)[rows, :].T.astype(bf).reshape(16, 128, HPC * HD).transpose(1, 0, 2)),
            "wvt": np.ascontiguousarray(np.asarray(wv, np.float32)[rows, :].T.astype(bf).reshape(16, 128, HPC * HD).transpose(1, 0, 2)),
            "wot": wot,
            "cgrid": cg,
            "sgrid": sg,
            "masks": mk,
        })
    return in_maps


def assemble(results):
    out = np.empty((B, S, D), np.float32)
    for c in range(N_CORES):
        r = results[c]["out"]  # [D, B*SCHUNK]
        for b in range(B):
            out[b, c * SCHUNK:(c + 1) * SCHUNK, :] = (
                r[:, b * SCHUNK:(b + 1) * SCHUNK].T
            )
    return out


_NC_CACHE = []


def kernel(**inputs):
    """Full-input distributed attention on 8 TRN2 NeuronCores.

    Takes the unsharded inputs (x, freqs_cos, freqs_sin, wq, wk, wv, wo) as
    numpy float32 arrays, runs the SPMD bass kernel on cores 0-7, and
    returns the full [B, S, D] float32 output.
    """
    from concourse.bass_utils import run_bass_kernel_spmd

    if not _NC_CACHE:
        _NC_CACHE.append(build())
    nc = _NC_CACHE[0]
    in_maps = prep_inputs(
        x=inputs["x"],
        freqs_cos=inputs["freqs_cos"],
        freqs_sin=inputs["freqs_sin"],
        wq=inputs["wq"],
        wk=inputs["wk"],
        wv=inputs["wv"],
        wo=inputs["wo"],
    )
    res = run_bass_kernel_spmd(nc, in_maps, CORE_IDS, trace=False)
    return assemble(res.results)



# revision 5
# speedup vs baseline: 1.0268x; 1.0268x over previous
"""Distributed multi-head causal attention for TRN2, 8 NeuronCores.

Strategy (tensor-parallel over heads + AllToAll re-shard for the output
projection):
  - Each core owns 2 of the 16 heads. It computes Q,K,V projections for its
    heads over the full sequence (both batches), applies RoPE, and computes
    causal softmax(QK^T/sqrt(hd)) @ V for its heads.
  - Everything on-chip is laid out TRANSPOSED: qT/kT are [hd, B*S], scores are
    [k, q], attention output is [hd, q].  This avoids all transposes:
      scoresT = kT_block.T @ qT        (lhsT=kT block, rhs=qT)
      aoT     = v_block.T  @ pT        (lhsT=v natural [k,hd], rhs=pT [k,q])
    Softmax denominator (sum over k = partition axis) comes from a ones-matmul
    (lhsT=ones [128,128]) that also broadcasts the sum across partitions.
    exp() is computed WITHOUT max subtraction (max |score| ~ 6, safe in f32).
  - Four AllToAlls (one per (batch, head)) swap head-sharding for
    sequence-sharding; each fires as soon as that (batch, head)'s attention
    output is stored, overlapping the collective with the remaining attention
    compute.  Each core ends with all 16 heads for its 256-position slice of
    each batch, then computes its slice of the wo projection:
    outT = woT_chunk.T @ attnT (output transposed; host transposes back).
  - Compute dtype: bf16 matmul operands, f32 PSUM accumulation, f32 softmax.

Host-side prep casts inputs to bf16 and pre-transposes x/wo; host-side
assembly transposes/concats per-core outputs.  No host arithmetic.
"""
import math

import ml_dtypes
import numpy as np

import concourse.bass as bass
import concourse.mybir as mybir
from concourse import bacc
from concourse.tile import TileContext

F32 = mybir.dt.float32
BF16 = mybir.dt.bfloat16

N_CORES = 8
CORE_IDS = list(range(N_CORES))
B = 2
S = 2048
D = 2048
H = 16
HD = 128  # head dim
HPC = H // N_CORES  # heads per core = 2
BS = B * S  # 4096
NB = S // 512  # 4 q-free-blocks per batch
NK = S // 128  # 16 k-blocks per batch
SCHUNK = S // N_CORES  # 256 positions per core per batch
INV_SQRT_HD = 1.0 / math.sqrt(HD)

# stream_shuffle mask: swap adjacent partitions within each 32-group
PAIR_SWAP = [i ^ 1 for i in range(32)]


def build():
    nc = bacc.Bacc(None, num_devices=N_CORES)

    # x pre-transposed/chunked: [half, j2, p, i, n] fully contiguous per
    # (half, j2) so startup DMAs are large sequential reads.
    xt = nc.declare_dram_parameter("xt", [2 * B, 2, 128, 16, 512], BF16, isOutput=False)
    # per-head weight chunks: [h, p, i, hd] contiguous per head.
    wqt = nc.declare_dram_parameter("wqt", [HPC, 128, 16, HD], BF16, isOutput=False)
    wkt = nc.declare_dram_parameter("wkt", [HPC, 128, 16, HD], BF16, isOutput=False)
    wvt = nc.declare_dram_parameter("wvt", [HPC, 128, 16, HD], BF16, isOutput=False)
    wot = nc.declare_dram_parameter("wot", [128, 16, D], BF16, isOutput=False)
    cgrid = nc.declare_dram_parameter("cgrid", [HD, S], F32, isOutput=False)
    sgrid = nc.declare_dram_parameter("sgrid", [HD, S], F32, isOutput=False)
    masks = nc.declare_dram_parameter("masks", [HD, 2, 1024], BF16, isOutput=False)
    out_ext = nc.declare_dram_parameter("out", [D, B * SCHUNK], F32, isOutput=True)

    bnc_in = [
        [nc.dram_tensor(f"bounce_in{b}_{h}", [N_CORES, HD, SCHUNK], BF16)
         for h in range(HPC)]
        for b in range(B)
    ]
    bnc_out = [
        [nc.dram_tensor(f"bounce_out{b}_{h}", [N_CORES, HD, SCHUNK], BF16)
         for h in range(HPC)]
        for b in range(B)
    ]

    bar_in = nc.dram_tensor("bar_in", [1], F32)
    bar_out = nc.dram_tensor("bar_out", [N_CORES], F32, addr_space="Shared")

    with TileContext(nc) as tc:
        with (
            tc.tile_pool(name="persist", bufs=1) as persist,
            tc.tile_pool(name="tmp", bufs=4) as tmp,
        ):
            # ---------------- persistent SBUF tensors ----------------
            mask_sb = persist.tile([128, 2, 1024], BF16, tag="mask")
            ones_sb = persist.tile([128, 128], BF16, tag="ones")
            nc.vector.memset(ones_sb, 1.0)

            # qT/kT per head: [hd=128, BS] bf16 (post-RoPE).
            # v per head: [128, BS] bf16, chunk ik at cols [128*ik,128*(ik+1))
            # holding v rows (k) on partitions, hd on free.
            q_sb = [persist.tile([128, BS], BF16, tag=f"q{h}", name=f"q_sb{h}") for h in range(HPC)]
            k_sb = [persist.tile([128, BS], BF16, tag=f"k{h}", name=f"k_sb{h}") for h in range(HPC)]
            v_sb = [persist.tile([128, BS], BF16, tag=f"v{h}", name=f"v_sb{h}") for h in range(HPC)]

            # ---------------- phase 1: QKV projections + RoPE ----------------
            with (
                tc.tile_pool(name="p1w", bufs=1) as p1w,
                tc.tile_pool(name="xt_pool", bufs=2) as xt_pool,
                tc.tile_pool(name="p1psum", bufs=1, space="PSUM") as p1psum,
                tc.tile_pool(name="p1psumv", bufs=2, space="PSUM") as p1psumv,
            ):
                wq_sb = p1w.tile([128, 16, HPC * HD], BF16, tag="wq")
                wk_sb = p1w.tile([128, 16, HPC * HD], BF16, tag="wk")
                wv_sb = p1w.tile([128, 16, HPC * HD], BF16, tag="wv")
                cg_sb = p1w.tile([128, S], F32, tag="cg")
                sg_sb = p1w.tile([128, S], F32, tag="sg")

                # Startup DMA priority: the very first QK psum group needs
                # wq(h=0) + xt(half0, j2=0) only.  Spread the critical chunks
                # across the three DMA-capable trigger queues (gpsimd, sync,
                # scalar); defer everything else.  xt chunks are interleaved
                # below in the loop (sync+scalar).
                xt00_sb = xt_pool.tile([128, 16, 512], BF16, tag="xt")
                nc.sync.dma_start(out=xt00_sb[:, 0:8, :], in_=xt[0, 0, :, 0:8, :])
                nc.scalar.dma_start(out=xt00_sb[:, 8:16, :], in_=xt[0, 0, :, 8:16, :])
                nc.gpsimd.dma_start(out=wq_sb[:, :, 0:HD], in_=wqt[0])
                # dummy AllGather: absorbs cross-core NEFF-launch skew early,
                # so the later AllToAlls see aligned peers
                nc.gpsimd.collective_compute(
                    "AllGather",
                    mybir.AluOpType.bypass,
                    replica_groups=[CORE_IDS],
                    ins=[bar_in[:]],
                    outs=[bar_out[:]],
                )
                nc.sync.dma_start(out=wq_sb[:, :, HD:2 * HD], in_=wqt[1])
                nc.gpsimd.dma_start(out=wk_sb[:, :, 0:HD], in_=wkt[0])
                nc.sync.dma_start(out=wk_sb[:, :, HD:2 * HD], in_=wkt[1])
                # RoPE grids (first halves needed right after first psum group)
                nc.gpsimd.dma_start(out=cg_sb[:, 0:1024], in_=cgrid[:, 0:1024])
                nc.scalar.dma_start(out=sg_sb[:, 0:1024], in_=sgrid[:, 0:1024])
                nc.gpsimd.dma_start(out=wv_sb[:, :, 0:HD], in_=wvt[0])
                nc.scalar.dma_start(out=wv_sb[:, :, HD:2 * HD], in_=wvt[1])
                nc.gpsimd.dma_start(out=cg_sb[:, 1024:2048], in_=cgrid[:, 1024:2048])
                nc.gpsimd.dma_start(out=sg_sb[:, 1024:2048], in_=sgrid[:, 1024:2048])
                nc.gpsimd.dma_start(out=mask_sb, in_=masks[:, :, :])

                for half in range(2 * B):  # half-batches of 1024 positions
                    b, hf = divmod(half, 2)
                    for j2 in range(2):
                        coff = b * S + hf * 1024 + j2 * 512  # col off in [D, BS]
                        poff = hf * 1024 + j2 * 512  # position offset (grids)
                        if half == 0 and j2 == 0:
                            xt_sb = xt00_sb
                        else:
                            xt_sb = xt_pool.tile([128, 16, 512], BF16, tag="xt")
                            nc.sync.dma_start(
                                out=xt_sb[:, 0:8, :], in_=xt[half, j2, :, 0:8, :]
                            )
                            nc.scalar.dma_start(
                                out=xt_sb[:, 8:16, :], in_=xt[half, j2, :, 8:16, :]
                            )

                        # Q, K for both heads: psum [hd, 512] accum over d_in
                        ps = {}
                        for kind, w in (("q", wq_sb), ("k", wk_sb)):
                            for h in range(HPC):
                                p = p1psum.tile([128, 512], F32, tag=f"qk{kind}{h}")
                                ps[(kind, h)] = p
                                for i in range(16):
                                    nc.tensor.matmul(
                                        p,
                                        w[:, i, h * HD:(h + 1) * HD],
                                        xt_sb[:, i, :],
                                        start=(i == 0),
                                        stop=(i == 15),
                                    )
                        # RoPE: out = t*cos + pairswap(t)*sin_signed (DVE only)
                        gcol = slice(poff, poff + 512)
                        ocol = slice(coff, coff + 512)
                        for kind, dst in (("q", q_sb), ("k", k_sb)):
                            for h in range(HPC):
                                p = ps[(kind, h)]
                                m1 = tmp.tile([128, 512], F32, tag="rope_m1")
                                nc.vector.tensor_mul(m1, p, cg_sb[:, gcol])
                                sh = tmp.tile([128, 512], F32, tag="rope_sh")
                                nc.vector.stream_shuffle(sh, p, PAIR_SWAP)
                                nc.vector.tensor_mul(sh, sh, sg_sb[:, gcol])
                                nc.vector.tensor_add(dst[h][:, ocol], m1, sh)

                        # V for both heads: psum [s=128, 2*HD] accum over d_in
                        for s2 in range(4):
                            pv = p1psumv.tile([128, HPC * HD], F32, tag="v")
                            for i in range(16):
                                nc.tensor.matmul(
                                    pv,
                                    xt_sb[:, i, s2 * 128:(s2 + 1) * 128],
                                    wv_sb[:, i, :],
                                    start=(i == 0),
                                    stop=(i == 15),
                                )
                            sc = hf * 8 + j2 * 4 + s2
                            ccol = slice((b * NK + sc) * 128, (b * NK + sc + 1) * 128)
                            for h in range(HPC):
                                nc.scalar.copy(
                                    out=v_sb[h][:, ccol], in_=pv[:, h * HD:(h + 1) * HD]
                                )

            # ---------------- phases 2+3 SBUF pool ----------------
            with (
                tc.tile_pool(name="p23", bufs=1) as p23,
                tc.tile_pool(name="ptile", bufs=6) as ptile,
            ):
                wo_sb = p23.tile([128, 16, D], BF16, tag="wo")
                nc.scalar.dma_start(out=wo_sb, in_=wot[:, :, :])

                # ---------------- phase 2: attention (batch-major) ----------------
                with tc.tile_pool(name="p2psum", bufs=2, space="PSUM") as p2psum:
                    for b in range(B):
                        for h in range(HPC):
                            for jq in range(NB):
                                po = p2psum.tile([128, 512], F32, tag="pv", bufs=2)
                                pden = p2psum.tile([128, 512], F32, tag="den", bufs=2)
                                nkb = 4 * jq + 4  # causal: k-blocks 0..4jq+3
                                qcol = slice(b * S + jq * 512, b * S + (jq + 1) * 512)
                                # q-columns 256-511 only (high diagonal pair)
                                qcol_hi = slice(b * S + jq * 512 + 256, b * S + (jq + 1) * 512)
                                d_prev = None
                                for e in range(nkb // 2):  # k-block pairs
                                    hi = e == 2 * jq + 1  # high diagonal pair:
                                    # blocks 4jq+2/4jq+3 only reach q >= 256
                                    w = 256 if hi else 512
                                    psc = p2psum.tile([128, 2 * w], F32, tag="sc", name="psc")
                                    for u in range(2):
                                        ik = 2 * e + u
                                        nc.tensor.matmul(
                                            psc[:, u * w:(u + 1) * w],
                                            k_sb[h][:, b * S + ik * 128: b * S + (ik + 1) * 128],
                                            q_sb[h][:, qcol_hi if hi else qcol],
                                            start=True,
                                            stop=True,
                                        )
                                    p_sb = ptile.tile([128, 2 * w], BF16, tag="p", name="p_sb")
                                    nc.scalar.activation(
                                        out=p_sb,
                                        in_=psc,
                                        func=mybir.ActivationFunctionType.Exp,
                                        scale=INV_SQRT_HD,
                                    )
                                    if e >= 2 * jq:  # diagonal pair: causal 0/1 mask
                                        nc.vector.tensor_mul(
                                            p_sb, p_sb, mask_sb[:, e - 2 * jq, 0:2 * w]
                                        )
                                    for u in range(2):
                                        ik = 2 * e + u
                                        vcol = slice((b * NK + ik) * 128, (b * NK + ik + 1) * 128)
                                        nc.tensor.matmul(
                                            po[:, 256:512] if hi else po,
                                            v_sb[h][:, vcol], p_sb[:, u * w:(u + 1) * w],
                                            start=(ik == 0), stop=(ik == nkb - 1),
                                            skip_group_check=True,
                                        )
                                    d_sb = tmp.tile([128, 512], BF16, tag="dpair")
                                    nc.vector.tensor_add(
                                        d_sb[:, 0:w], p_sb[:, 0:w], p_sb[:, w:2 * w]
                                    )
                                    if e < 2 * jq and e % 2 == 0:
                                        d_prev = d_sb  # defer: pair up with next
                                        continue
                                    if e < 2 * jq:  # odd off-diagonal: fold 2 pairs
                                        dd = tmp.tile([128, 512], BF16, tag="dquad")
                                        nc.vector.tensor_add(dd, d_prev, d_sb)
                                        d_sb = dd
                                    nc.tensor.matmul(
                                        pden[:, 256:512] if hi else pden,
                                        ones_sb, d_sb[:, 0:w],
                                        start=(e == (1 if jq > 0 else 0) and not hi),
                                        stop=hi,
                                        skip_group_check=True,
                                    )
                                recip = tmp.tile([128, 512], F32, tag="recip")
                                nc.vector.reciprocal_approx_fast(out=recip, in_=pden)
                                ao = tmp.tile([128, 512], BF16, tag="ao")
                                nc.vector.tensor_mul(ao, po, recip)
                                for u in range(2):
                                    nc.gpsimd.dma_start(
                                        out=bnc_in[b][h][2 * jq + u, :, :],
                                        in_=ao[:, u * 256:(u + 1) * 256],
                                    )
                            # fire this (batch, head)'s AllToAll immediately;
                            # overlaps with the remaining attention compute
                            nc.gpsimd.collective_compute(
                                "AllToAll",
                                mybir.AluOpType.bypass,
                                replica_groups=[CORE_IDS],
                                ins=[bnc_in[b][h][:, :, :]],
                                outs=[bnc_out[b][h][:, :, :]],
                            )

                # ---------------- phase 3: output projection ----------------
                # gather DMAs on the (idle) sync queue, as soon as each
                # AllToAll lands; g{b}{h} holds heads of parity h for batch b.
                g_sb = [
                    [p23.tile([128, N_CORES, SCHUNK], BF16, tag=f"g{b}{h}",
                              name=f"g_sb{b}{h}") for h in range(HPC)]
                    for b in range(B)
                ]
                for b in range(B):
                    for h in range(HPC):
                        nc.sync.dma_start(
                            out=g_sb[b][h],
                            in_=bnc_out[b][h].rearrange("j p n -> p j n", p=128),
                        )
                with tc.tile_pool(name="p3psum", bufs=2, space="PSUM") as p3psum:
                    for b in range(B):
                        for m in range(16):
                            pw = p3psum.tile([128, SCHUNK], F32, tag="wo", bufs=2)
                            for i in range(16):
                                nc.tensor.matmul(
                                    pw,
                                    wo_sb[:, i, m * 128:(m + 1) * 128],
                                    g_sb[b][i % 2][:, i // 2, :],
                                    start=(i == 0),
                                    stop=(i == 15),
                                )
                            o_sb = tmp.tile([128, SCHUNK], F32, tag="o")
                            nc.vector.tensor_copy(out=o_sb, in_=pw)
                            nc.scalar.dma_start(
                                out=out_ext[m * 128:(m + 1) * 128, b * SCHUNK:(b + 1) * SCHUNK],
                                in_=o_sb,
                            )

    nc.compile()
    return nc


def prep_inputs(x, freqs_cos, freqs_sin, wq, wk, wv, wo):
    """Host-side shard prep. Returns in_maps (list of 8 dicts)."""
    bf = ml_dtypes.bfloat16
    x = np.asarray(x, dtype=np.float32)
    xtf = x.reshape(BS, D).T.astype(bf)  # [D, BS]
    # partition-major pre-chunk: [half, j2, p, i, n] -> fully sequential DMAs
    xt = np.ascontiguousarray(
        xtf.reshape(16, 128, 2 * B, 2, 512).transpose(2, 3, 1, 0, 4)
    )
    wot = np.ascontiguousarray(np.asarray(wo, np.float32).T.astype(bf).reshape(16, 128, D).transpose(1, 0, 2))
    cos = np.asarray(freqs_cos, np.float32)
    sin = np.asarray(freqs_sin, np.float32)
    cg = np.empty((HD, S), np.float32)
    sg = np.empty((HD, S), np.float32)
    cg[0::2] = cos.T
    cg[1::2] = cos.T
    sg[0::2] = -sin.T
    sg[1::2] = sin.T
    mk4 = np.zeros((4, HD, 512), np.float32)
    for t in range(4):
        kp = np.arange(HD)[:, None]
        qf = np.arange(512)[None, :]
        mk4[t] = (128 * t + kp <= qf).astype(np.float32)
    # mk[0]: low diagonal pair (blocks t0,t1) over full 512 q-cols;
    # mk[1][:, :512]: high pair (t2,t3) restricted to q-cols 256-511
    mk = np.zeros((2, HD, 1024), np.float32)
    mk[0][:, 0:512] = mk4[0]
    mk[0][:, 512:1024] = mk4[1]
    mk[1][:, 0:256] = mk4[2][:, 256:512]
    mk[1][:, 256:512] = mk4[3][:, 256:512]
    mk = np.ascontiguousarray(mk.astype(bf).transpose(1, 0, 2))

    def wchunks(w, rows):
        # [D_out rows slice].T -> [2048, 256] -> per-head [h, 128, 16, 128]
        wt = np.asarray(w, np.float32)[rows, :].T.astype(bf)  # [D, 256]
        return np.ascontiguousarray(
            wt.reshape(16, 128, HPC, HD).transpose(2, 1, 0, 3)
        )

    in_maps = []
    for c in range(N_CORES):
        rows = slice(c * HPC * HD, (c + 1) * HPC * HD)
        in_maps.append({
            "xt": xt,
            "wqt": wchunks(wq, rows),
            "wkt": wchunks(wk, rows),
            "wvt": wchunks(wv, rows),
            "wot": wot,
            "cgrid": cg,
            "sgrid": sg,
            "masks": mk,
        })
    return in_maps


def assemble(results):
    out = np.empty((B, S, D), np.float32)
    for c in range(N_CORES):
        r = results[c]["out"]  # [D, B*SCHUNK]
        for b in range(B):
            out[b, c * SCHUNK:(c + 1) * SCHUNK, :] = (
                r[:, b * SCHUNK:(b + 1) * SCHUNK].T
            )
    return out


_NC_CACHE = []


def kernel(**inputs):
    """Full-input distributed attention on 8 TRN2 NeuronCores.

    Takes the unsharded inputs (x, freqs_cos, freqs_sin, wq, wk, wv, wo) as
    numpy float32 arrays, runs the SPMD bass kernel on cores 0-7, and
    returns the full [B, S, D] float32 output.
    """
    from concourse.bass_utils import run_bass_kernel_spmd

    if not _NC_CACHE:
        _NC_CACHE.append(build())
    nc = _NC_CACHE[0]
    in_maps = prep_inputs(
        x=inputs["x"],
        freqs_cos=inputs["freqs_cos"],
        freqs_sin=inputs["freqs_sin"],
        wq=inputs["wq"],
        wk=inputs["wk"],
        wv=inputs["wv"],
        wo=inputs["wo"],
    )
    res = run_bass_kernel_spmd(nc, in_maps, CORE_IDS, trace=False)
    return assemble(res.results)


# revision 11
# speedup vs baseline: 1.0793x; 1.0511x over previous
"""Distributed multi-head causal attention for TRN2, 8 NeuronCores.

Strategy (tensor-parallel over heads + AllToAll re-shard for the output
projection):
  - Each core owns 2 of the 16 heads. It computes Q,K,V projections for its
    heads over the full sequence (both batches), applies RoPE, and computes
    causal softmax(QK^T/sqrt(hd)) @ V for its heads.
  - Everything on-chip is laid out TRANSPOSED: qT/kT are [hd, B*S], scores are
    [k, q], attention output is [hd, q].  This avoids all transposes:
      scoresT = kT_block.T @ qT        (lhsT=kT block, rhs=qT)
      aoT     = v_block.T  @ pT        (lhsT=v natural [k,hd], rhs=pT [k,q])
    Softmax denominator (sum over k = partition axis) comes from a ones-matmul
    (lhsT=ones [128,128]) that also broadcasts the sum across partitions.
    exp() is computed WITHOUT max subtraction (max |score| ~ 6, safe in f32).
  - Four AllToAlls (one per (batch, head)) swap head-sharding for
    sequence-sharding; each fires as soon as that (batch, head)'s attention
    output is stored, overlapping the collective with the remaining attention
    compute.  Each core ends with all 16 heads for its 256-position slice of
    each batch, then computes its slice of the wo projection:
    outT = woT_chunk.T @ attnT (output transposed; host transposes back).
  - Compute dtype: bf16 matmul operands, f32 PSUM accumulation, f32 softmax.

Host-side prep casts inputs to bf16 and pre-transposes x/wo; host-side
assembly transposes/concats per-core outputs.  No host arithmetic.
"""
import math

import ml_dtypes
import numpy as np

import concourse.bass as bass
import concourse.mybir as mybir
from concourse import bacc
from concourse.tile import TileContext

F32 = mybir.dt.float32
BF16 = mybir.dt.bfloat16

N_CORES = 8
CORE_IDS = list(range(N_CORES))
B = 2
S = 2048
D = 2048
H = 16
HD = 128  # head dim
HPC = H // N_CORES  # heads per core = 2
BS = B * S  # 4096
NB = S // 512  # 4 q-free-blocks per batch
NK = S // 128  # 16 k-blocks per batch
SCHUNK = S // N_CORES  # 256 positions per core per batch
INV_SQRT_HD = 1.0 / math.sqrt(HD)

# stream_shuffle mask: swap adjacent partitions within each 32-group
PAIR_SWAP = [i ^ 1 for i in range(32)]


def build():
    nc = bacc.Bacc(None, num_devices=N_CORES)

    # x pre-transposed/chunked: [half, j2, p, i, n] fully contiguous per
    # (half, j2) so startup DMAs are large sequential reads.
    xt = nc.declare_dram_parameter("xt", [2 * B, 2, 128, 16, 512], BF16, isOutput=False)
    # per-head weight chunks: [h, p, i, hd] contiguous per head.
    wqt = nc.declare_dram_parameter("wqt", [HPC, 128, 16, HD], BF16, isOutput=False)
    wkt = nc.declare_dram_parameter("wkt", [HPC, 128, 16, HD], BF16, isOutput=False)
    wvt = nc.declare_dram_parameter("wvt", [HPC, 128, 16, HD], BF16, isOutput=False)
    wot = nc.declare_dram_parameter("wot", [128, 16, D], BF16, isOutput=False)
    cgrid = nc.declare_dram_parameter("cgrid", [HD, S], F32, isOutput=False)
    sgrid = nc.declare_dram_parameter("sgrid", [HD, S], F32, isOutput=False)
    masks = nc.declare_dram_parameter("masks", [HD, 2, 1024], BF16, isOutput=False)
    out_ext = nc.declare_dram_parameter("out", [D, B * SCHUNK], F32, isOutput=True)

    bnc_in = [
        [nc.dram_tensor(f"bounce_in{b}_{h}", [N_CORES, HD, SCHUNK], BF16)
         for h in range(HPC)]
        for b in range(B)
    ]
    bnc_out = [
        [nc.dram_tensor(f"bounce_out{b}_{h}", [N_CORES, HD, SCHUNK], BF16)
         for h in range(HPC)]
        for b in range(B)
    ]

    bar_in = nc.dram_tensor("bar_in", [1], F32)
    bar_out = nc.dram_tensor("bar_out", [N_CORES], F32, addr_space="Shared")

    with TileContext(nc) as tc:
        with (
            tc.tile_pool(name="persist", bufs=1) as persist,
            tc.tile_pool(name="tmp", bufs=4) as tmp,
        ):
            # ---------------- persistent SBUF tensors ----------------
            mask_sb = persist.tile([128, 2, 1024], BF16, tag="mask")
            ones_sb = persist.tile([128, 128], BF16, tag="ones")
            nc.vector.memset(ones_sb, 1.0)

            # qT/kT per head: [hd=128, BS] bf16 (post-RoPE).
            # v per head: [128, BS] bf16, chunk ik at cols [128*ik,128*(ik+1))
            # holding v rows (k) on partitions, hd on free.
            q_sb = [persist.tile([128, BS], BF16, tag=f"q{h}", name=f"q_sb{h}") for h in range(HPC)]
            k_sb = [persist.tile([128, BS], BF16, tag=f"k{h}", name=f"k_sb{h}") for h in range(HPC)]
            v_sb = [persist.tile([128, BS], BF16, tag=f"v{h}", name=f"v_sb{h}") for h in range(HPC)]

            # ---------------- phase 1: QKV projections + RoPE ----------------
            with (
                tc.tile_pool(name="p1w", bufs=1) as p1w,
                tc.tile_pool(name="xt_pool", bufs=2) as xt_pool,
                tc.tile_pool(name="p1psum", bufs=1, space="PSUM") as p1psum,
                tc.tile_pool(name="p1psumv", bufs=2, space="PSUM") as p1psumv,
            ):
                wq_sb = p1w.tile([128, 16, HPC * HD], BF16, tag="wq")
                wk_sb = p1w.tile([128, 16, HPC * HD], BF16, tag="wk")
                wv_sb = p1w.tile([128, 16, HPC * HD], BF16, tag="wv")
                cg_sb = p1w.tile([128, S], F32, tag="cg")
                sg_sb = p1w.tile([128, S], F32, tag="sg")

                # Startup DMA priority: the sync+scalar queues carry ONLY the
                # xt stream (so nothing backs it up); gpsimd carries weights +
                # grids + mask in exact demand order.  The first xt chunk is
                # split 4 ways i-ascending so the first QK accumulation can
                # begin as soon as the first i-chunks land.
                xt00_sb = xt_pool.tile([128, 16, 512], BF16, tag="xt")
                nc.sync.dma_start(out=xt00_sb[:, 0:4, :], in_=xt[0, 0, :, 0:4, :])
                nc.scalar.dma_start(out=xt00_sb[:, 4:8, :], in_=xt[0, 0, :, 4:8, :])
                nc.sync.dma_start(out=xt00_sb[:, 8:12, :], in_=xt[0, 0, :, 8:12, :])
                nc.scalar.dma_start(out=xt00_sb[:, 12:16, :], in_=xt[0, 0, :, 12:16, :])
                nc.gpsimd.dma_start(out=wq_sb[:, :, 0:HD], in_=wqt[0])
                # dummy AllGather: absorbs cross-core NEFF-launch skew early,
                # so the later AllToAlls see aligned peers
                nc.gpsimd.collective_compute(
                    "AllGather",
                    mybir.AluOpType.bypass,
                    replica_groups=[CORE_IDS],
                    ins=[bar_in[:]],
                    outs=[bar_out[:]],
                )
                nc.gpsimd.dma_start(out=wq_sb[:, :, HD:2 * HD], in_=wqt[1])
                nc.gpsimd.dma_start(out=wk_sb[:, :, 0:HD], in_=wkt[0])
                nc.gpsimd.dma_start(out=wk_sb[:, :, HD:2 * HD], in_=wkt[1])
                # RoPE grids in demand order: first 512 cols feed the
                # first RoPE block; the rest can trickle in later.
                nc.gpsimd.dma_start(out=cg_sb[:, 0:512], in_=cgrid[:, 0:512])
                nc.gpsimd.dma_start(out=sg_sb[:, 0:512], in_=sgrid[:, 0:512])
                nc.gpsimd.dma_start(out=wv_sb[:, :, 0:HD], in_=wvt[0])
                nc.gpsimd.dma_start(out=wv_sb[:, :, HD:2 * HD], in_=wvt[1])
                nc.gpsimd.dma_start(out=cg_sb[:, 512:1024], in_=cgrid[:, 512:1024])
                nc.gpsimd.dma_start(out=sg_sb[:, 512:1024], in_=sgrid[:, 512:1024])
                nc.gpsimd.dma_start(out=cg_sb[:, 1024:2048], in_=cgrid[:, 1024:2048])
                nc.gpsimd.dma_start(out=sg_sb[:, 1024:2048], in_=sgrid[:, 1024:2048])
                nc.gpsimd.dma_start(out=mask_sb, in_=masks[:, :, :])

                for half in range(2 * B):  # half-batches of 1024 positions
                    b, hf = divmod(half, 2)
                    for j2 in range(2):
                        coff = b * S + hf * 1024 + j2 * 512  # col off in [D, BS]
                        poff = hf * 1024 + j2 * 512  # position offset (grids)
                        if half == 0 and j2 == 0:
                            xt_sb = xt00_sb
                        else:
                            xt_sb = xt_pool.tile([128, 16, 512], BF16, tag="xt")
                            nc.sync.dma_start(
                                out=xt_sb[:, 0:8, :], in_=xt[half, j2, :, 0:8, :]
                            )
                            nc.scalar.dma_start(
                                out=xt_sb[:, 8:16, :], in_=xt[half, j2, :, 8:16, :]
                            )

                        # Q, K for both heads: psum [hd, 512] accum over d_in
                        ps = {}
                        for kind, w in (("q", wq_sb), ("k", wk_sb)):
                            for h in range(HPC):
                                p = p1psum.tile([128, 512], F32, tag=f"qk{kind}{h}")
                                ps[(kind, h)] = p
                                for i in range(16):
                                    nc.tensor.matmul(
                                        p,
                                        w[:, i, h * HD:(h + 1) * HD],
                                        xt_sb[:, i, :],
                                        start=(i == 0),
                                        stop=(i == 15),
                                    )
                        # RoPE: out = t*cos + pairswap(t)*sin_signed (DVE only)
                        gcol = slice(poff, poff + 512)
                        ocol = slice(coff, coff + 512)
                        for kind, dst in (("q", q_sb), ("k", k_sb)):
                            for h in range(HPC):
                                p = ps[(kind, h)]
                                m1 = tmp.tile([128, 512], F32, tag="rope_m1")
                                nc.vector.tensor_mul(m1, p, cg_sb[:, gcol])
                                sh = tmp.tile([128, 512], F32, tag="rope_sh")
                                nc.vector.stream_shuffle(sh, p, PAIR_SWAP)
                                nc.vector.tensor_mul(sh, sh, sg_sb[:, gcol])
                                nc.vector.tensor_add(dst[h][:, ocol], m1, sh)

                        # V for both heads: psum [s=128, 2*HD] accum over d_in
                        for s2 in range(4):
                            pv = p1psumv.tile([128, HPC * HD], F32, tag="v")
                            for i in range(16):
                                nc.tensor.matmul(
                                    pv,
                                    xt_sb[:, i, s2 * 128:(s2 + 1) * 128],
                                    wv_sb[:, i, :],
                                    start=(i == 0),
                                    stop=(i == 15),
                                )
                            sc = hf * 8 + j2 * 4 + s2
                            ccol = slice((b * NK + sc) * 128, (b * NK + sc + 1) * 128)
                            for h in range(HPC):
                                nc.scalar.copy(
                                    out=v_sb[h][:, ccol], in_=pv[:, h * HD:(h + 1) * HD]
                                )

            # ---------------- phases 2+3 SBUF pool ----------------
            with (
                tc.tile_pool(name="p23", bufs=1) as p23,
                tc.tile_pool(name="ptile", bufs=6) as ptile,
            ):
                wo_sb = p23.tile([128, 16, D], BF16, tag="wo")
                nc.scalar.dma_start(out=wo_sb, in_=wot[:, :, :])

                # ---------------- phase 2: attention (batch-major) ----------------
                with tc.tile_pool(name="p2psum", bufs=2, space="PSUM") as p2psum:
                    for b in range(B):
                        for h in range(HPC):
                            for jq in range(NB):
                                po = p2psum.tile([128, 512], F32, tag="pv", bufs=2)
                                pden = p2psum.tile([128, 512], F32, tag="den", bufs=2)
                                nkb = 4 * jq + 4  # causal: k-blocks 0..4jq+3
                                qcol = slice(b * S + jq * 512, b * S + (jq + 1) * 512)
                                # q-columns 256-511 only (high diagonal pair)
                                qcol_hi = slice(b * S + jq * 512 + 256, b * S + (jq + 1) * 512)
                                d_prev = None
                                for e in range(nkb // 2):  # k-block pairs
                                    hi = e == 2 * jq + 1  # high diagonal pair:
                                    # blocks 4jq+2/4jq+3 only reach q >= 256
                                    w = 256 if hi else 512
                                    psc = p2psum.tile([128, 2 * w], F32, tag="sc", name="psc")
                                    for u in range(2):
                                        ik = 2 * e + u
                                        nc.tensor.matmul(
                                            psc[:, u * w:(u + 1) * w],
                                            k_sb[h][:, b * S + ik * 128: b * S + (ik + 1) * 128],
                                            q_sb[h][:, qcol_hi if hi else qcol],
                                            start=True,
                                            stop=True,
                                        )
                                    p_sb = ptile.tile([128, 2 * w], BF16, tag="p", name="p_sb")
                                    nc.scalar.activation(
                                        out=p_sb,
                                        in_=psc,
                                        func=mybir.ActivationFunctionType.Exp,
                                        scale=INV_SQRT_HD,
                                    )
                                    if e >= 2 * jq:  # diagonal pair: causal 0/1 mask
                                        nc.vector.tensor_mul(
                                            p_sb, p_sb, mask_sb[:, e - 2 * jq, 0:2 * w]
                                        )
                                    for u in range(2):
                                        ik = 2 * e + u
                                        vcol = slice((b * NK + ik) * 128, (b * NK + ik + 1) * 128)
                                        nc.tensor.matmul(
                                            po[:, 256:512] if hi else po,
                                            v_sb[h][:, vcol], p_sb[:, u * w:(u + 1) * w],
                                            start=(ik == 0), stop=(ik == nkb - 1),
                                            skip_group_check=True,
                                        )
                                    d_sb = tmp.tile([128, 512], BF16, tag="dpair")
                                    nc.vector.tensor_add(
                                        d_sb[:, 0:w], p_sb[:, 0:w], p_sb[:, w:2 * w]
                                    )
                                    if e < 2 * jq and e % 2 == 0:
                                        d_prev = d_sb  # defer: pair up with next
                                        continue
                                    if e < 2 * jq:  # odd off-diagonal: fold 2 pairs
                                        dd = tmp.tile([128, 512], BF16, tag="dquad")
                                        nc.vector.tensor_add(dd, d_prev, d_sb)
                                        d_sb = dd
                                    nc.tensor.matmul(
                                        pden[:, 256:512] if hi else pden,
                                        ones_sb, d_sb[:, 0:w],
                                        start=(e == (1 if jq > 0 else 0) and not hi),
                                        stop=hi,
                                        skip_group_check=True,
                                    )
                                recip = tmp.tile([128, 512], F32, tag="recip")
                                nc.vector.reciprocal_approx_fast(out=recip, in_=pden)
                                ao = tmp.tile([128, 512], BF16, tag="ao")
                                nc.vector.tensor_mul(ao, po, recip)
                                for u in range(2):
                                    nc.gpsimd.dma_start(
                                        out=bnc_in[b][h][2 * jq + u, :, :],
                                        in_=ao[:, u * 256:(u + 1) * 256],
                                    )
                            # fire this (batch, head)'s AllToAll immediately;
                            # overlaps with the remaining attention compute
                            nc.gpsimd.collective_compute(
                                "AllToAll",
                                mybir.AluOpType.bypass,
                                replica_groups=[CORE_IDS],
                                ins=[bnc_in[b][h][:, :, :]],
                                outs=[bnc_out[b][h][:, :, :]],
                            )

                # ---------------- phase 3: output projection ----------------
                # gather DMAs on the (idle) sync queue, as soon as each
                # AllToAll lands; g{b}{h} holds heads of parity h for batch b.
                g_sb = [
                    [p23.tile([128, N_CORES, SCHUNK], BF16, tag=f"g{b}{h}",
                              name=f"g_sb{b}{h}") for h in range(HPC)]
                    for b in range(B)
                ]
                for b in range(B):
                    for h in range(HPC):
                        nc.sync.dma_start(
                            out=g_sb[b][h],
                            in_=bnc_out[b][h].rearrange("j p n -> p j n", p=128),
                        )
                # two passes over head parity: pass 0 uses only the h=0
                # AllToAll results (which land earlier), so ~half the output
                # projection can run before the last AllToAll completes.
                # NOTE: matmul start=True resets the whole PSUM *bank*, so
                # every concurrently-open accumulation group needs its own
                # bank: 8 groups of [128,256] at a time, two m-groups.
                with tc.tile_pool(name="p3psum", bufs=1, space="PSUM") as p3psum:
                    for b in range(B):
                        for mg in range(2):
                            pws = [
                                p3psum.tile([128, SCHUNK], F32, tag=f"wo{j}",
                                            name=f"pw{b}_{mg}_{j}")
                                for j in range(8)
                            ]
                            for par in range(2):
                                for j in range(8):
                                    m = mg * 8 + j
                                    pw = pws[j]
                                    for i2 in range(8):
                                        nc.tensor.matmul(
                                            pw,
                                            wo_sb[:, 2 * i2 + par, m * 128:(m + 1) * 128],
                                            g_sb[b][par][:, i2, :],
                                            start=(par == 0 and i2 == 0),
                                            stop=(par == 1 and i2 == 7),
                                            skip_group_check=True,
                                        )
                                    if par == 1:
                                        o_sb = tmp.tile([128, SCHUNK], F32, tag="o")
                                        nc.vector.tensor_copy(out=o_sb, in_=pw)
                                        nc.scalar.dma_start(
                                            out=out_ext[m * 128:(m + 1) * 128, b * SCHUNK:(b + 1) * SCHUNK],
                                            in_=o_sb,
                                        )

    nc.compile()
    return nc


def prep_inputs(x, freqs_cos, freqs_sin, wq, wk, wv, wo):
    """Host-side shard prep. Returns in_maps (list of 8 dicts)."""
    bf = ml_dtypes.bfloat16
    x = np.asarray(x, dtype=np.float32)
    xtf = x.reshape(BS, D).T.astype(bf)  # [D, BS]
    # partition-major pre-chunk: [half, j2, p, i, n] -> fully sequential DMAs
    xt = np.ascontiguousarray(
        xtf.reshape(16, 128, 2 * B, 2, 512).transpose(2, 3, 1, 0, 4)
    )
    wot = np.ascontiguousarray(np.asarray(wo, np.float32).T.astype(bf).reshape(16, 128, D).transpose(1, 0, 2))
    cos = np.asarray(freqs_cos, np.float32)
    sin = np.asarray(freqs_sin, np.float32)
    cg = np.empty((HD, S), np.float32)
    sg = np.empty((HD, S), np.float32)
    cg[0::2] = cos.T
    cg[1::2] = cos.T
    sg[0::2] = -sin.T
    sg[1::2] = sin.T
    mk4 = np.zeros((4, HD, 512), np.float32)
    for t in range(4):
        kp = np.arange(HD)[:, None]
        qf = np.arange(512)[None, :]
        mk4[t] = (128 * t + kp <= qf).astype(np.float32)
    # mk[0]: low diagonal pair (blocks t0,t1) over full 512 q-cols;
    # mk[1][:, :512]: high pair (t2,t3) restricted to q-cols 256-511
    mk = np.zeros((2, HD, 1024), np.float32)
    mk[0][:, 0:512] = mk4[0]
    mk[0][:, 512:1024] = mk4[1]
    mk[1][:, 0:256] = mk4[2][:, 256:512]
    mk[1][:, 256:512] = mk4[3][:, 256:512]
    mk = np.ascontiguousarray(mk.astype(bf).transpose(1, 0, 2))

    def wchunks(w, rows):
        # [D_out rows slice].T -> [2048, 256] -> per-head [h, 128, 16, 128]
        wt = np.asarray(w, np.float32)[rows, :].T.astype(bf)  # [D, 256]
        return np.ascontiguousarray(
            wt.reshape(16, 128, HPC, HD).transpose(2, 1, 0, 3)
        )

    in_maps = []
    for c in range(N_CORES):
        rows = slice(c * HPC * HD, (c + 1) * HPC * HD)
        in_maps.append({
            "xt": xt,
            "wqt": wchunks(wq, rows),
            "wkt": wchunks(wk, rows),
            "wvt": wchunks(wv, rows),
            "wot": wot,
            "cgrid": cg,
            "sgrid": sg,
            "masks": mk,
        })
    return in_maps


def assemble(results):
    out = np.empty((B, S, D), np.float32)
    for c in range(N_CORES):
        r = results[c]["out"]  # [D, B*SCHUNK]
        for b in range(B):
            out[b, c * SCHUNK:(c + 1) * SCHUNK, :] = (
                r[:, b * SCHUNK:(b + 1) * SCHUNK].T
            )
    return out


_NC_CACHE = []


def kernel(**inputs):
    """Full-input distributed attention on 8 TRN2 NeuronCores.

    Takes the unsharded inputs (x, freqs_cos, freqs_sin, wq, wk, wv, wo) as
    numpy float32 arrays, runs the SPMD bass kernel on cores 0-7, and
    returns the full [B, S, D] float32 output.
    """
    from concourse.bass_utils import run_bass_kernel_spmd

    if not _NC_CACHE:
        _NC_CACHE.append(build())
    nc = _NC_CACHE[0]
    in_maps = prep_inputs(
        x=inputs["x"],
        freqs_cos=inputs["freqs_cos"],
        freqs_sin=inputs["freqs_sin"],
        wq=inputs["wq"],
        wk=inputs["wk"],
        wv=inputs["wv"],
        wo=inputs["wo"],
    )
    res = run_bass_kernel_spmd(nc, in_maps, CORE_IDS, trace=False)
    return assemble(res.results)
